# revision 11
# baseline (speedup 1.0000x reference)
"""Trainium2 Bass kernel for nn_ASMLoc_Base (topk_masking).

B=32,T=1024,D=2048,NCLS=21. Data-parallel over batch: 4 videos per core x 8 cores.
Per core:
  phase W: conv_w f32 -> bf16 cast -> DRAM; xbar-transpose halves into SBUF as W^T.
  phase X: input f32 -> bf16 -> DRAM; xbar-transpose to x^T [din,t] (zero-padded t).
  conv:    emb[dout,t] = relu(sum_k W_k^T.T @ x^T(shift k) + b), bf16 matmuls, fp32 psum.
  stage2:  Z[24,t] = cmb @ emb (rows 0..20 cls, 21/22 att, 23 att-diff), PE-transpose
           128-chunks -> per-t softmaxes (cas/fg/bg) + temp_att; fg/bg cas packed
           into [v*21+c, t] tiles for the top-k path.
  topk:    per-row binary search (30 iters) for the k-th largest, masked sum, mean,
           then per-video softmax over classes -> fg_cls/bg_cls.
"""

import os
import sys

for _p in ("/opt/trn_rl_repo", "/root/.axon_site/_ro/trn_rl_repo"):
    if os.path.isdir(_p) and _p not in sys.path:
        sys.path.insert(0, _p)

import numpy as np

import concourse.bass as bass
import concourse.tile as tile
from concourse import bacc, mybir
from concourse.bass_utils import run_bass_kernel_spmd

B, T, D, NCLS = 32, 1024, 2048, 21
NCORES = 8
BL = B // NCORES            # videos per core
FGK, BGK = T // 8, T // 3   # 128, 341
NDT = D // 128              # 16 din (and dout) tiles
NCR = NCLS + 3              # 21 cls + att0 + att1 + attdiff = 24
F32 = mybir.dt.float32
BF16 = mybir.dt.bfloat16
AX = mybir.AxisListType
OP = mybir.AluOpType
AF = mybir.ActivationFunctionType

N_ITER = 30  # topk binary-search iterations


def build_nc():
    nc = bacc.Bacc("TRN2", target_bir_lowering=False, debug=False)

    xi = nc.declare_dram_parameter("x", [BL, T, D], F32, isOutput=False)
    cw = nc.declare_dram_parameter("conv_w", [D, D, 3], F32, isOutput=False)
    cbias = nc.declare_dram_parameter("cbias", [128, NDT], F32, isOutput=False)
    cmbt = nc.declare_dram_parameter("cmbt", [D, NCR], F32, isOutput=False)
    zbias = nc.declare_dram_parameter("zbias", [NCR, 1], F32, isOutput=False)
    ident = nc.declare_dram_parameter("ident", [128, 128], F32, isOutput=False)

    o_fg_cls = nc.declare_dram_parameter("fg_cls", [BL, NCLS], F32, isOutput=True)
    o_bg_cls = nc.declare_dram_parameter("bg_cls", [BL, NCLS], F32, isOutput=True)
    o_ta = nc.declare_dram_parameter("temp_att", [BL, T, 2], F32, isOutput=True)
    o_cas = nc.declare_dram_parameter("cas_sm", [BL, T, NCLS], F32, isOutput=True)
    o_fg = nc.declare_dram_parameter("fg_sm", [BL, T, NCLS], F32, isOutput=True)
    o_bg = nc.declare_dram_parameter("bg_sm", [BL, T, NCLS], F32, isOutput=True)

    w_bf = nc.dram_tensor("w_bf", [3, D, D], BF16)
    x_bf = nc.dram_tensor("x_bf", [BL, T, D], BF16)
    xt_d = nc.dram_tensor("xt_d", [BL, D, T + 2], BF16)
    emb_bf = nc.dram_tensor("emb_bf", [BL, D, T], BF16)

    with tile.TileContext(nc) as tc, \
         tc.tile_pool(name="persist", bufs=1) as persist, \
         tc.tile_pool(name="psZ", bufs=1, space="PSUM") as psZ, \
         tc.tile_pool(name="psZT", bufs=1, space="PSUM") as psZT, \
         tc.tile_pool(name="psBC", bufs=1, space="PSUM") as psBC:

        # ---------------- persistent small tiles ----------------
        cb_sb = persist.tile([128, NDT], F32)
        nc.gpsimd.dma_start(cb_sb[:], cbias[:])
        zb_sb = persist.tile([NCR, 1], F32)
        nc.gpsimd.dma_start(zb_sb[:], zbias[:])
        ident_f = persist.tile([128, 128], F32)
        nc.gpsimd.dma_start(ident_f[:], ident[:])
        cmbT = persist.tile([128, NDT, NCR], BF16)
        ktA = persist.tile([128, 1], F32)
        ktB = persist.tile([128, 1], F32)
        kiA = persist.tile([128, 1], F32)
        kiB = persist.tile([128, 1], F32)
        nc.vector.memset(ktA[:], float(FGK))
        nc.vector.memset(ktB[:], float(BGK))
        nc.vector.memset(kiA[:], 1.0 / FGK)
        nc.vector.memset(kiB[:], 1.0 / BGK)
        ones21 = persist.tile([1, NCLS], F32)
        nc.vector.memset(ones21[:], 1.0)
        zeroK = persist.tile([128, NDT, 1], BF16)
        nc.vector.memset(zeroK[:], 0.0)

        packA = persist.tile([128, T], F32)   # fg cas, row v*21+c
        packB = persist.tile([128, T], F32)   # bg cas, row v*21+c
        nc.vector.memset(packA[:], 0.0)
        nc.vector.memset(packB[:], 0.0)

        # cmbt f32 -> bf16 tiles [128, 24] per din-tile (single load + cast)
        with tc.tile_pool(name="cmb_stage", bufs=1) as cmb_stage:
            cf = cmb_stage.tile([128, NDT, NCR], F32)
            nc.gpsimd.dma_start(
                cf[:], cmbt.rearrange("(a p) c -> p a c", p=128))
            nc.vector.tensor_copy(cmbT[:], cf[:])

        # ---------------- phase W: conv_w cast to bf16 in DRAM ----------------
        with tc.tile_pool(name="wf", bufs=2) as poolWf, \
             tc.tile_pool(name="wb", bufs=4) as poolWb:
            for ot in range(NDT):
                wf = poolWf.tile([128, D, 3], F32)
                nc.sync.dma_start(wf[:], cw[ot * 128:(ot + 1) * 128, :, :])
                for k in range(3):
                    wb = poolWb.tile([128, D], BF16)
                    nc.vector.tensor_copy(wb[:], wf[:, :, k])
                    nc.gpsimd.dma_start(w_bf[k, ot * 128:(ot + 1) * 128, :], wb[:])

        # ---------------- phase X: x cast + transpose to xt_d ----------------
        with tc.tile_pool(name="xf", bufs=2) as poolXf, \
             tc.tile_pool(name="xb", bufs=4) as poolXb, \
             tc.tile_pool(name="xt", bufs=4) as poolXT:
            for v in range(BL):
                for tt in range(T // 128):
                    xf = poolXf.tile([128, D], F32)
                    nc.sync.dma_start(xf[:], xi[v, tt * 128:(tt + 1) * 128, :])
                    xb = poolXb.tile([128, D], BF16)
                    nc.vector.tensor_copy(xb[:], xf[:])
                    nc.gpsimd.dma_start(x_bf[v, tt * 128:(tt + 1) * 128, :], xb[:])
                for dt in range(NDT):
                    xt = poolXT.tile([128, T], BF16)
                    nc.sync.dma_start(
                        xt[:],
                        x_bf[v, :, dt * 128:(dt + 1) * 128],
                        transpose=True,
                    )
                    nc.gpsimd.dma_start(
                        xt_d[v, dt * 128:(dt + 1) * 128, 1:T + 1], xt[:])
                # zero the two pad columns (0 and T+1) in DRAM, once per video
                nc.gpsimd.dma_start(
                    xt_d[v, :, 0:1].rearrange("(a p) c -> p a c", p=128),
                    zeroK[:])
                nc.gpsimd.dma_start(
                    xt_d[v, :, T + 1:T + 2].rearrange("(a p) c -> p a c", p=128),
                    zeroK[:])

        # ---------------- conv + stage2 ----------------
        def stage2(v, poolER, poolZS, poolS, poolSm, poolOut):
            """emit stage-2 for video v (emb_bf[v] ready)."""
            for c in range(2):
                pz = psZ.tile([NCR, 512], F32)
                for og in range(NDT):
                    er = poolER.tile([128, 512], BF16)
                    nc.sync.dma_start(
                        er[:],
                        emb_bf[v, og * 128:(og + 1) * 128, c * 512:(c + 1) * 512])
                    nc.tensor.matmul(
                        pz[:], cmbT[:, og, :], er[:],
                        start=(og == 0), stop=(og == NDT - 1))
                z_sb = poolZS.tile([NCR, 512], F32)
                nc.vector.tensor_scalar_add(z_sb[:], pz[:], zb_sb[:, 0:1])

                # fg/bg att scalars as [1,512] rows for the pack-multiply
                # (compute-engine partition base must be 0/32/64/96, so run the
                # sigmoid on the whole tile and DMA row 23 out)
                s23 = poolS.tile([NCR, 1024], F32, tag="s23")
                nc.scalar.activation(s23[:, 0:512], z_sb[:], AF.Sigmoid)
                nc.scalar.activation(s23[:, 512:1024], z_sb[:],
                                     AF.Sigmoid, scale=-1.0)
                s_f = poolS.tile([1, 512], F32, tag="s_f")
                s_g = poolS.tile([1, 512], F32, tag="s_g")
                nc.gpsimd.dma_start(s_f[:], s23[23:24, 0:512])
                nc.gpsimd.dma_start(s_g[:], s23[23:24, 512:1024])

                bc_f = psBC.tile([NCLS, 512], F32)
                bc_g = psBC.tile([NCLS, 512], F32)
                nc.tensor.matmul(bc_f[:], ones21[:], s_f[:])
                nc.tensor.matmul(bc_g[:], ones21[:], s_g[:])

                fcs = poolS.tile([NCLS, 512], F32, tag="cas_stage")
                gcs = poolS.tile([NCLS, 512], F32, tag="cas_stage")
                nc.vector.tensor_mul(fcs[:], z_sb[0:NCLS, :], bc_f[:])
                nc.vector.tensor_mul(gcs[:], z_sb[0:NCLS, :], bc_g[:])
                nc.gpsimd.dma_start(
                    packA[v * NCLS:(v + 1) * NCLS, c * 512:(c + 1) * 512], fcs[:])
                nc.gpsimd.dma_start(
                    packB[v * NCLS:(v + 1) * NCLS, c * 512:(c + 1) * 512], gcs[:])

                for q in range(4):
                    tq = c * 512 + q * 128
                    pzt = psZT.tile([128, NCR], F32)
                    nc.tensor.transpose(
                        pzt[:], z_sb[:, q * 128:(q + 1) * 128],
                        ident_f[0:NCR, 0:NCR])
                    ta = poolSm.tile([128, 2], F32, tag="ta")
                    nc.scalar.activation(ta[:, 0:1], pzt[:, 23:24], AF.Sigmoid)
                    nc.scalar.activation(ta[:, 1:2], pzt[:, 23:24], AF.Sigmoid,
                                         scale=-1.0)
                    nc.gpsimd.dma_start(o_ta[v, tq:tq + 128, :], ta[:])

                    for which, scol, dst in (
                            ("cas", None, o_cas), ("fg", 0, o_fg), ("bg", 1, o_bg)):
                        if scol is None:
                            logits = pzt[:, 0:NCLS]
                        else:
                            lg = poolSm.tile([128, NCLS], F32, tag="lg")
                            nc.vector.tensor_scalar_mul(
                                lg[:], pzt[:, 0:NCLS], ta[:, scol:scol + 1])
                            logits = lg[:]
                        nm = poolSm.tile([128, 1], F32, tag="nm")
                        nc.vector.tensor_reduce(
                            nm[:], logits, axis=AX.X, op=OP.max, negate=True)
                        ex = poolSm.tile([128, NCLS], F32, tag="ex")
                        sm = poolSm.tile([128, 1], F32, tag="sm")
                        nc.scalar.activation(ex[:], logits, AF.Exp,
                                             bias=nm[:, 0:1], accum_out=sm[:, 0:1])
                        rc = poolSm.tile([128, 1], F32, tag="rc")
                        nc.vector.reciprocal(rc[:], sm[:])
                        oo = poolOut.tile([128, NCLS], F32, tag="oo")
                        nc.vector.tensor_scalar_mul(oo[:], ex[:], rc[:, 0:1])
                        nc.gpsimd.dma_start(dst[v, tq:tq + 128, :], oo[:])

        with tc.tile_pool(name="wT", bufs=1) as poolWT, \
             tc.tile_pool(name="xc", bufs=2) as poolXC, \
             tc.tile_pool(name="psC", bufs=3, space="PSUM") as poolPS, \
             tc.tile_pool(name="embo", bufs=4) as poolEmb, \
             tc.tile_pool(name="er", bufs=6) as poolER, \
             tc.tile_pool(name="zs", bufs=2) as poolZS, \
             tc.tile_pool(name="s2s", bufs=2) as poolS, \
             tc.tile_pool(name="sm", bufs=4) as poolSm, \
             tc.tile_pool(name="smo", bufs=6) as poolOut:

            for h in range(2):
                wT = poolWT.tile([128, 3 * NDT * 1024], BF16, tag="wT")
                for k in range(3):
                    for dt in range(NDT):
                        nc.sync.dma_start(
                            wT[:, (k * NDT + dt) * 1024:(k * NDT + dt + 1) * 1024],
                            w_bf[k, h * 1024:(h + 1) * 1024,
                                 dt * 128:(dt + 1) * 128],
                            transpose=True,
                        )
                for v in range(BL):
                    for c in range(2):
                        xc = poolXC.tile([128, NDT, 514], BF16, tag="xc")
                        for dt in range(NDT):
                            nc.sync.dma_start(
                                xc[:, dt, :],
                                xt_d[v, dt * 128:(dt + 1) * 128,
                                     c * 512:c * 512 + 514])
                        for ot in range(8):
                            ps = poolPS.tile([128, 512], F32)
                            for dt in range(NDT):
                                for k in range(3):
                                    base = (k * NDT + dt) * 1024 + ot * 128
                                    nc.tensor.matmul(
                                        ps[:],
                                        wT[:, base:base + 128],
                                        xc[:, dt, k:k + 512],
                                        start=(dt == 0 and k == 0),
                                        stop=(dt == NDT - 1 and k == 2),
                                    )
                            emb_t = poolEmb.tile([128, 512], BF16, tag="embo")
                            nc.scalar.activation(
                                emb_t[:], ps[:], AF.Relu,
                                bias=cb_sb[:, h * 8 + ot:h * 8 + ot + 1])
                            nc.gpsimd.dma_start(
                                emb_bf[v, (h * 8 + ot) * 128:(h * 8 + ot + 1) * 128,
                                       c * 512:(c + 1) * 512],
                                emb_t[:])
                    if h == 1:
                        stage2(v, poolER, poolZS, poolS, poolSm, poolOut)

            # ---------------- topk + class softmax ----------------
            with tc.tile_pool(name="tk", bufs=1) as tk, \
                 tc.tile_pool(name="tks", bufs=2) as tks:
                for pack, kt, ki, out_t in ((packA, ktA, kiA, o_fg_cls),
                                            (packB, ktB, kiB, o_bg_cls)):
                    lo = tk.tile([128, 1], F32, tag=f"lo{out_t.name}")
                    hi = tk.tile([128, 1], F32, tag=f"hi{out_t.name}")
                    mid = tk.tile([128, 1], F32, tag=f"mid{out_t.name}")
                    cnt = tk.tile([128, 1], F32, tag=f"cnt{out_t.name}")
                    ge = tk.tile([128, 1], mybir.dt.int32, tag=f"ge{out_t.name}")
                    lt = tk.tile([128, 1], mybir.dt.int32, tag=f"lt{out_t.name}")
                    nc.vector.tensor_reduce(lo[:], pack[:], axis=AX.X, op=OP.min)
                    nc.vector.tensor_reduce(hi[:], pack[:], axis=AX.X, op=OP.max)
                    # hi += (hi-lo)*1e-6 + 1e-12 so cnt(x>=hi) < k strictly
                    nc.vector.tensor_sub(mid[:], hi[:], lo[:])
                    nc.vector.tensor_scalar(
                        out=mid[:], in0=mid[:], scalar1=1e-6, scalar2=1e-12,
                        op0=OP.mult, op1=OP.add)
                    nc.vector.tensor_add(hi[:], hi[:], mid[:])
                    for it in range(N_ITER):
                        nc.vector.tensor_scalar(
                            out=mid[:], in0=lo[:], scalar1=hi[:, 0:1], scalar2=0.5,
                            op0=OP.add, op1=OP.mult)
                        scr = tks.tile([128, T], F32, tag="scr")
                        nc.vector.tensor_scalar(
                            out=scr[:], in0=pack[:], scalar1=mid[:, 0:1],
                            scalar2=None, op0=OP.is_ge, op1=OP.add,
                            accum_out=cnt[:, 0:1])
                        nc.vector.tensor_scalar(
                            out=ge[:], in0=cnt[:], scalar1=kt[:, 0:1], scalar2=None,
                            op0=OP.is_ge)
                        nc.vector.tensor_scalar(
                            out=lt[:], in0=cnt[:], scalar1=kt[:, 0:1], scalar2=None,
                            op0=OP.is_lt)
                        nc.vector.copy_predicated(lo[:], ge[:], mid[:])
                        nc.vector.copy_predicated(hi[:], lt[:], mid[:])
                    # final: cnt(x>=lo), sum(x * (x>=lo))
                    scr = tks.tile([128, T], F32, tag="scr")
                    nc.vector.tensor_scalar(
                        out=scr[:], in0=pack[:], scalar1=lo[:, 0:1], scalar2=None,
                        op0=OP.is_ge, op1=OP.add, accum_out=cnt[:, 0:1])
                    ssum = tk.tile([128, 1], F32, tag=f"ss{out_t.name}")
                    scr2 = tks.tile([128, T], F32, tag="scr")
                    nc.vector.scalar_tensor_tensor(
                        out=scr2[:], in0=pack[:], scalar=lo[:, 0:1], in1=pack[:],
                        op0=OP.is_ge, op1=OP.mult, accum_out=ssum[:, 0:1])
                    # mean = (ssum - (cnt-k)*lo) / k
                    nc.vector.tensor_sub(cnt[:], cnt[:], kt[:])
                    nc.vector.tensor_mul(cnt[:], cnt[:], lo[:])
                    nc.vector.tensor_sub(ssum[:], ssum[:], cnt[:])
                    nc.vector.tensor_mul(ssum[:], ssum[:], ki[:])
                    # rearrange [v*21+c, 1] -> [v, c] and softmax over classes
                    mv = tk.tile([BL, NCLS], F32, tag=f"mv{out_t.name}")
                    nc.gpsimd.dma_start(mv[:], ssum[0:BL * NCLS, 0:1])
                    nm = tk.tile([BL, 1], F32, tag=f"nm{out_t.name}")
                    nc.vector.tensor_reduce(nm[:], mv[:], axis=AX.X, op=OP.max,
                                            negate=True)
                    ex = tk.tile([BL, NCLS], F32, tag=f"ex{out_t.name}")
                    sm = tk.tile([BL, 1], F32, tag=f"sm{out_t.name}")
                    nc.scalar.activation(ex[:], mv[:], AF.Exp, bias=nm[:, 0:1],
                                         accum_out=sm[:, 0:1])
                    rc = tk.tile([BL, 1], F32, tag=f"rc{out_t.name}")
                    nc.vector.reciprocal(rc[:], sm[:])
                    oo = tk.tile([BL, NCLS], F32, tag=f"oo{out_t.name}")
                    nc.vector.tensor_scalar_mul(oo[:], ex[:], rc[:, 0:1])
                    nc.gpsimd.dma_start(out_t[:], oo[:])

    nc.compile()
    return nc


_NC_CACHE = None


def _get_nc():
    global _NC_CACHE
    if _NC_CACHE is None:
        _NC_CACHE = build_nc()
    return _NC_CACHE


def make_in_maps(input_feature, conv_w, conv_b, att_w, att_b, cls_w, cls_b):
    input_feature = np.ascontiguousarray(input_feature, dtype=np.float32)
    conv_w = np.ascontiguousarray(conv_w, dtype=np.float32)
    conv_b = np.asarray(conv_b, dtype=np.float32)
    att_w = np.asarray(att_w, dtype=np.float32).reshape(2, D)
    att_b = np.asarray(att_b, dtype=np.float32)
    cls_w = np.asarray(cls_w, dtype=np.float32)
    cls_b = np.asarray(cls_b, dtype=np.float32)

    cmb = np.concatenate(
        [cls_w, att_w, (att_w[0] - att_w[1])[None, :]], axis=0)  # [24, D]
    cmbt = np.ascontiguousarray(cmb.T)  # [D, 24]
    zbias = np.concatenate(
        [cls_b, att_b, np.array([att_b[0] - att_b[1]], np.float32)]
    ).reshape(NCR, 1).astype(np.float32)
    cbias = np.ascontiguousarray(conv_b.reshape(NDT, 128).T)  # [128, 16]
    idm = np.eye(128, dtype=np.float32)

    in_maps = []
    for i in range(NCORES):
        in_maps.append({
            "x": np.ascontiguousarray(input_feature[i * BL:(i + 1) * BL]),
            "conv_w": conv_w,
            "cbias": cbias,
            "cmbt": cmbt,
            "zbias": zbias,
            "ident": idm,
        })
    return in_maps


def gather(rs):
    fg_cls = np.concatenate([r["fg_cls"] for r in rs], axis=0)
    bg_cls = np.concatenate([r["bg_cls"] for r in rs], axis=0)
    temp_att = np.concatenate([r["temp_att"] for r in rs], axis=0)
    cas_sm = np.concatenate([r["cas_sm"] for r in rs], axis=0)
    fg_sm = np.concatenate([r["fg_sm"] for r in rs], axis=0)
    bg_sm = np.concatenate([r["bg_sm"] for r in rs], axis=0)
    return (fg_cls, bg_cls, temp_att, cas_sm, fg_sm, bg_sm)


def kernel(input_feature, conv_w, conv_b, att_w, att_b, cls_w, cls_b):
    nc = _get_nc()
    in_maps = make_in_maps(input_feature, conv_w, conv_b, att_w, att_b,
                           cls_w, cls_b)
    res = run_bass_kernel_spmd(nc, in_maps, list(range(NCORES)))
    return gather(res.results)


# revision 16
# speedup vs baseline: 1.3069x; 1.3069x over previous
"""Trainium2 Bass kernel for nn_ASMLoc_Base (topk_masking).

B=32,T=1024,D=2048,NCLS=21. Data-parallel over batch: 4 videos per core x 8 cores.
Per core:
  prep:   gpsimd cast-DMAs f32->bf16 (DRAM->DRAM), xbar DMA-transposes to build
          W^T [din,dout] (SBUF, per dout-half) and x^T [din,t] (DRAM, padded).
  conv:   emb[dout,t] = relu(sum_k W_k^T.T @ x^T(shift k) + b): 48 accumulating
          bf16 128x128x512 matmuls per psum tile, ReLU+bias on ScalarE.
  stage2: Z[24,t] = cmb @ emb (rows 0..20 cls, 21/22 att, 23 att-diff), PE
          transposes -> per-t softmaxes (cas/fg/bg, ScalarE exp only) + temp_att;
          fg/bg cas packed per video [42, t] for top-k.
  topk:   per-video binary search (24 iters) for the k-th largest, masked sum,
          mean, then per-video softmax over classes -> fg_cls/bg_cls.
Emission order interleaves weight/input prep with conv so PE starts early.
"""

import os
import sys

for _p in ("/opt/trn_rl_repo", "/root/.axon_site/_ro/trn_rl_repo"):
    if os.path.isdir(_p) and _p not in sys.path:
        sys.path.insert(0, _p)

import numpy as np

import concourse.bass as bass
import concourse.tile as tile
from concourse import bacc, mybir
from concourse.bass_utils import run_bass_kernel_spmd

B, T, D, NCLS = 32, 1024, 2048, 21
NCORES = 8
BL = B // NCORES            # videos per core
FGK, BGK = T // 8, T // 3   # 128, 341
NDT = D // 128              # 16 din (and dout) tiles
NCR = NCLS + 3              # 21 cls + att0 + att1 + attdiff = 24
F32 = mybir.dt.float32
BF16 = mybir.dt.bfloat16
AX = mybir.AxisListType
OP = mybir.AluOpType
AF = mybir.ActivationFunctionType

N_ITER = 24  # topk binary-search iterations


def build_nc():
    nc = bacc.Bacc("TRN2", target_bir_lowering=False, debug=False)

    xi = nc.declare_dram_parameter("x", [BL, T, D], F32, isOutput=False)
    cw = nc.declare_dram_parameter("conv_w", [D, D, 3], F32, isOutput=False)
    cbias = nc.declare_dram_parameter("cbias", [128, NDT], F32, isOutput=False)
    cmbt = nc.declare_dram_parameter("cmbt", [D, NCR], F32, isOutput=False)
    zbias = nc.declare_dram_parameter("zbias", [NCR, 1], F32, isOutput=False)
    ident = nc.declare_dram_parameter("ident", [128, 128], F32, isOutput=False)
    kvec = nc.declare_dram_parameter("kvec", [42, 1], F32, isOutput=False)
    kinv = nc.declare_dram_parameter("kinv", [42, 1], F32, isOutput=False)

    o_fg_cls = nc.declare_dram_parameter("fg_cls", [BL, NCLS], F32, isOutput=True)
    o_bg_cls = nc.declare_dram_parameter("bg_cls", [BL, NCLS], F32, isOutput=True)
    o_ta = nc.declare_dram_parameter("temp_att", [BL, T, 2], F32, isOutput=True)
    o_cas = nc.declare_dram_parameter("cas_sm", [BL, T, NCLS], F32, isOutput=True)
    o_fg = nc.declare_dram_parameter("fg_sm", [BL, T, NCLS], F32, isOutput=True)
    o_bg = nc.declare_dram_parameter("bg_sm", [BL, T, NCLS], F32, isOutput=True)

    w_bf = nc.dram_tensor("w_bf", [3, D, D], BF16)     # cast + k-deinterleaved
    x_bf = nc.dram_tensor("x_bf", [BL, T, D], BF16)
    xt_d = nc.dram_tensor("xt_d", [BL, D, T + 2], BF16)
    emb_bf = nc.dram_tensor("emb_bf", [BL, D, T], BF16)

    with tile.TileContext(nc) as tc, \
         tc.tile_pool(name="persist", bufs=1) as persist, \
         tc.tile_pool(name="psZ", bufs=1, space="PSUM") as psZ, \
         tc.tile_pool(name="psZT", bufs=1, space="PSUM") as psZT, \
         tc.tile_pool(name="psBC", bufs=1, space="PSUM") as psBC:

        # ---------------- persistent small tiles ----------------
        cb_sb = persist.tile([128, NDT], F32)
        nc.gpsimd.dma_start(cb_sb[:], cbias[:])
        zb_sb = persist.tile([NCR, 1], F32)
        nc.gpsimd.dma_start(zb_sb[:], zbias[:])
        ident_f = persist.tile([128, 128], F32)
        nc.gpsimd.dma_start(ident_f[:], ident[:])
        kt_sb = persist.tile([42, 1], F32)
        nc.gpsimd.dma_start(kt_sb[:], kvec[:])
        ki_sb = persist.tile([42, 1], F32)
        nc.gpsimd.dma_start(ki_sb[:], kinv[:])
        ones21 = persist.tile([1, NCLS], F32)
        nc.vector.memset(ones21[:], 1.0)
        zeroK = persist.tile([128, NDT, 1], BF16)
        nc.vector.memset(zeroK[:], 0.0)
        # cmbt f32 -> bf16 [128, dt, 24], cast in DMA
        cmbT = persist.tile([128, NDT, NCR], BF16)
        nc.gpsimd.dma_start(cmbT[:], cmbt.rearrange("(a p) c -> p a c", p=128))

        def emit_once(rep):
          with tc.tile_pool(name=f"wf{rep}", bufs=2) as poolWf, \
               tc.tile_pool(name=f"wb{rep}", bufs=2) as poolWb, \
               tc.tile_pool(name=f"xt{rep}", bufs=2) as poolXT, \
               tc.tile_pool(name=f"wT{rep}", bufs=2) as poolWT, \
               tc.tile_pool(name=f"xc{rep}", bufs=2) as poolXC, \
               tc.tile_pool(name=f"psC{rep}", bufs=3, space="PSUM") as poolPS, \
               tc.tile_pool(name=f"embo{rep}", bufs=4) as poolEmb, \
               tc.tile_pool(name=f"er{rep}", bufs=5) as poolER, \
               tc.tile_pool(name=f"zs{rep}", bufs=2) as poolZS, \
               tc.tile_pool(name=f"s2s{rep}", bufs=2) as poolS, \
               tc.tile_pool(name=f"sm{rep}", bufs=4) as poolSm, \
               tc.tile_pool(name=f"smo{rep}", bufs=6) as poolOut, \
               tc.tile_pool(name=f"tk{rep}", bufs=2) as tk, \
               tc.tile_pool(name=f"tks{rep}", bufs=1) as tks:

            def w_prep(q):
                # cast-load f32->bf16, k-deinterleave on DVE, store per-k
                for ot in range(q * 4, (q + 1) * 4):
                    wf = poolWf.tile([128, D, 3], BF16, tag="wf")
                    nc.gpsimd.dma_start(
                        wf[:], cw[ot * 128:(ot + 1) * 128, :, :])
                    for k in range(3):
                        wb = poolWb.tile([128, D], BF16, tag="wb")
                        nc.vector.tensor_copy(wb[:], wf[:, :, k])
                        nc.gpsimd.dma_start(
                            w_bf[k, ot * 128:(ot + 1) * 128, :], wb[:])

            def wT_load(q):
                # quarter of the dout dim: [din, 512] per (k, dt)
                wT = poolWT.tile([128, 3 * NDT * 512], BF16, tag="wT")
                for k in range(3):
                    for dt in range(NDT):
                        nc.sync.dma_start(
                            wT[:, (k * NDT + dt) * 512:(k * NDT + dt + 1) * 512],
                            w_bf[k, q * 512:(q + 1) * 512,
                                 dt * 128:(dt + 1) * 128],
                            transpose=True,
                        )
                return wT

            def x_prep(v):
                nc.gpsimd.dma_start(x_bf[v], xi[v])  # cast DMA DRAM->DRAM
                for dt in range(NDT):
                    xt = poolXT.tile([128, T], BF16, tag="xt")
                    nc.sync.dma_start(
                        xt[:], x_bf[v, :, dt * 128:(dt + 1) * 128],
                        transpose=True)
                    nc.gpsimd.dma_start(
                        xt_d[v, dt * 128:(dt + 1) * 128, 1:T + 1], xt[:])
                nc.gpsimd.dma_start(
                    xt_d[v, :, 0:1].rearrange("(a p) c -> p a c", p=128),
                    zeroK[:])
                nc.gpsimd.dma_start(
                    xt_d[v, :, T + 1:T + 2].rearrange("(a p) c -> p a c", p=128),
                    zeroK[:])

            def conv(q, v, wT):
                for c in range(2):
                    xc = poolXC.tile([128, NDT, 514], BF16, tag="xc")
                    for dt in range(NDT):
                        nc.sync.dma_start(
                            xc[:, dt, :],
                            xt_d[v, dt * 128:(dt + 1) * 128,
                                 c * 512:c * 512 + 514])
                    for ot in range(4):
                        ps = poolPS.tile([128, 512], F32)
                        for dt in range(NDT):
                            for k in range(3):
                                base = (k * NDT + dt) * 512 + ot * 128
                                nc.tensor.matmul(
                                    ps[:],
                                    wT[:, base:base + 128],
                                    xc[:, dt, k:k + 512],
                                    start=(dt == 0 and k == 0),
                                    stop=(dt == NDT - 1 and k == 2),
                                )
                        g = q * 4 + ot
                        emb_t = poolEmb.tile([128, 512], BF16, tag="embo")
                        nc.scalar.activation(
                            emb_t[:], ps[:], AF.Relu,
                            bias=cb_sb[:, g:g + 1])
                        nc.gpsimd.dma_start(
                            emb_bf[v, g * 128:(g + 1) * 128,
                                   c * 512:(c + 1) * 512],
                            emb_t[:])

            def stage2(v, pack):
                for c in range(2):
                    pz = psZ.tile([NCR, 512], F32)
                    for og in range(NDT):
                        er = poolER.tile([128, 512], BF16, tag="er")
                        nc.sync.dma_start(
                            er[:],
                            emb_bf[v, og * 128:(og + 1) * 128,
                                   c * 512:(c + 1) * 512])
                        nc.tensor.matmul(
                            pz[:], cmbT[:, og, :], er[:],
                            start=(og == 0), stop=(og == NDT - 1))
                    z_sb = poolZS.tile([NCR, 512], F32)
                    nc.vector.tensor_scalar_add(z_sb[:], pz[:], zb_sb[:, 0:1])

                    # fg/bg att rows via exp (stay in exp_and_others act set):
                    # fg = 1/(1+exp(-z23)), bg = 1/(1+exp(z23))
                    ef = poolS.tile([NCR, 512], F32, tag="e2")
                    eb = poolS.tile([NCR, 512], F32, tag="e2")
                    nc.scalar.activation(ef[:], z_sb[:], AF.Exp, scale=-1.0)
                    nc.scalar.activation(eb[:], z_sb[:], AF.Exp)
                    nc.vector.tensor_scalar_add(ef[:], ef[:], 1.0)
                    nc.vector.tensor_scalar_add(eb[:], eb[:], 1.0)
                    nc.vector.reciprocal(ef[:], ef[:])
                    nc.vector.reciprocal(eb[:], eb[:])
                    s_f = poolS.tile([1, 512], F32, tag="sfg")
                    s_g = poolS.tile([1, 512], F32, tag="sfg")
                    nc.gpsimd.dma_start(s_f[:], ef[23:24, :])
                    nc.gpsimd.dma_start(s_g[:], eb[23:24, :])

                    bc_f = psBC.tile([NCLS, 512], F32)
                    bc_g = psBC.tile([NCLS, 512], F32)
                    nc.tensor.matmul(bc_f[:], ones21[:], s_f[:])
                    nc.tensor.matmul(bc_g[:], ones21[:], s_g[:])

                    fcs = poolS.tile([NCLS, 512], F32, tag="cas_stage")
                    gcs = poolS.tile([NCLS, 512], F32, tag="cas_stage")
                    nc.vector.tensor_mul(fcs[:], z_sb[0:NCLS, :], bc_f[:])
                    nc.vector.tensor_mul(gcs[:], z_sb[0:NCLS, :], bc_g[:])
                    nc.gpsimd.dma_start(
                        pack[0:NCLS, c * 512:(c + 1) * 512], fcs[:])
                    nc.gpsimd.dma_start(
                        pack[NCLS:2 * NCLS, c * 512:(c + 1) * 512], gcs[:])

                    for q in range(4):
                        tq = c * 512 + q * 128
                        pzt = psZT.tile([128, NCR], F32)
                        nc.tensor.transpose(
                            pzt[:], z_sb[:, q * 128:(q + 1) * 128],
                            ident_f[0:NCR, 0:NCR])
                        ta = poolSm.tile([128, 2], F32, tag="ta")
                        tb = poolSm.tile([128, 2], F32, tag="tb")
                        nc.scalar.activation(tb[:, 0:1], pzt[:, 23:24], AF.Exp,
                                             scale=-1.0)
                        nc.scalar.activation(tb[:, 1:2], pzt[:, 23:24], AF.Exp)
                        nc.vector.tensor_scalar_add(tb[:], tb[:], 1.0)
                        nc.vector.reciprocal(ta[:], tb[:])
                        nc.gpsimd.dma_start(o_ta[v, tq:tq + 128, :], ta[:])

                        for scol, dst in ((None, o_cas), (0, o_fg), (1, o_bg)):
                            if scol is None:
                                logits = pzt[:, 0:NCLS]
                            else:
                                lg = poolSm.tile([128, NCLS], F32, tag="lg")
                                nc.vector.tensor_scalar_mul(
                                    lg[:], pzt[:, 0:NCLS],
                                    ta[:, scol:scol + 1])
                                logits = lg[:]
                            nm = poolSm.tile([128, 1], F32, tag="nm")
                            nc.vector.tensor_reduce(
                                nm[:], logits, axis=AX.X, op=OP.max, negate=True)
                            ex = poolSm.tile([128, NCLS], F32, tag="ex")
                            sm = poolSm.tile([128, 1], F32, tag="sm")
                            nc.scalar.activation(ex[:], logits, AF.Exp,
                                                 bias=nm[:, 0:1],
                                                 accum_out=sm[:, 0:1])
                            rc = poolSm.tile([128, 1], F32, tag="rc")
                            nc.vector.reciprocal(rc[:], sm[:])
                            oo = poolOut.tile([128, NCLS], F32, tag="oo")
                            nc.vector.tensor_scalar_mul(oo[:], ex[:], rc[:, 0:1])
                            nc.gpsimd.dma_start(dst[v, tq:tq + 128, :], oo[:])

            def topk(v, pack):
                lo = tk.tile([42, 1], F32, tag="lo")
                hi = tk.tile([42, 1], F32, tag="hi")
                mid = tk.tile([42, 1], F32, tag="mid")
                cnt = tk.tile([42, 1], F32, tag="cnt")
                ge = tk.tile([42, 1], mybir.dt.int32, tag="ge")
                lt = tk.tile([42, 1], mybir.dt.int32, tag="lt")
                nc.vector.tensor_reduce(lo[:], pack[:], axis=AX.X, op=OP.min)
                nc.vector.tensor_reduce(hi[:], pack[:], axis=AX.X, op=OP.max)
                # hi += (hi-lo)*1e-6 + 1e-12 so cnt(x>=hi) < k strictly
                nc.vector.tensor_sub(mid[:], hi[:], lo[:])
                nc.vector.tensor_scalar(
                    out=mid[:], in0=mid[:], scalar1=1e-6, scalar2=1e-12,
                    op0=OP.mult, op1=OP.add)
                nc.vector.tensor_add(hi[:], hi[:], mid[:])
                for it in range(N_ITER):
                    nc.vector.tensor_scalar(
                        out=mid[:], in0=lo[:], scalar1=hi[:, 0:1], scalar2=0.5,
                        op0=OP.add, op1=OP.mult)
                    scr = tks.tile([42, T], F32, tag="scr")
                    nc.vector.tensor_scalar(
                        out=scr[:], in0=pack[:], scalar1=mid[:, 0:1],
                        scalar2=None, op0=OP.is_ge, op1=OP.add,
                        accum_out=cnt[:, 0:1])
                    nc.vector.tensor_scalar(
                        out=ge[:], in0=cnt[:], scalar1=kt_sb[:, 0:1],
                        scalar2=None, op0=OP.is_ge)
                    nc.vector.tensor_scalar(
                        out=lt[:], in0=cnt[:], scalar1=kt_sb[:, 0:1],
                        scalar2=None, op0=OP.is_lt)
                    nc.vector.copy_predicated(lo[:], ge[:], mid[:])
                    nc.vector.copy_predicated(hi[:], lt[:], mid[:])
                scr = tks.tile([42, T], F32, tag="scr")
                nc.vector.tensor_scalar(
                    out=scr[:], in0=pack[:], scalar1=lo[:, 0:1], scalar2=None,
                    op0=OP.is_ge, op1=OP.add, accum_out=cnt[:, 0:1])
                ssum = tk.tile([42, 1], F32, tag="ss")
                scr2 = tks.tile([42, T], F32, tag="scr")
                nc.vector.scalar_tensor_tensor(
                    out=scr2[:], in0=pack[:], scalar=lo[:, 0:1], in1=pack[:],
                    op0=OP.is_ge, op1=OP.mult, accum_out=ssum[:, 0:1])
                # mean = (ssum - (cnt-k)*lo) / k
                nc.vector.tensor_sub(cnt[:], cnt[:], kt_sb[:])
                nc.vector.tensor_mul(cnt[:], cnt[:], lo[:])
                nc.vector.tensor_sub(ssum[:], ssum[:], cnt[:])
                nc.vector.tensor_mul(ssum[:], ssum[:], ki_sb[:])
                # [42,1] -> [2,21]; softmax over classes; rows: fg, bg
                mv = tk.tile([2, NCLS], F32, tag="mv")
                nc.gpsimd.dma_start(mv[:], ssum[:, 0:1])
                nm = tk.tile([2, 1], F32, tag="nm2")
                nc.vector.tensor_reduce(nm[:], mv[:], axis=AX.X, op=OP.max,
                                        negate=True)
                ex = tk.tile([2, NCLS], F32, tag="ex2")
                sm = tk.tile([2, 1], F32, tag="sm2")
                nc.scalar.activation(ex[:], mv[:], AF.Exp, bias=nm[:, 0:1],
                                     accum_out=sm[:, 0:1])
                rc = tk.tile([2, 1], F32, tag="rc2")
                nc.vector.reciprocal(rc[:], sm[:])
                oo = tk.tile([2, NCLS], F32, tag="oo2")
                nc.vector.tensor_scalar_mul(oo[:], ex[:], rc[:, 0:1])
                nc.gpsimd.dma_start(o_fg_cls[v:v + 1, :], oo[0:1, :])
                nc.gpsimd.dma_start(o_bg_cls[v:v + 1, :], oo[1:2, :])

            # ---------------- emission order ----------------
            w_prep(0)
            wTq = wT_load(0)
            x_prep(0)
            conv(0, 0, wTq)
            x_prep(1)
            w_prep(1)
            conv(0, 1, wTq)
            x_prep(2)
            conv(0, 2, wTq)
            x_prep(3)
            conv(0, 3, wTq)
            for q in range(1, 4):
                wTq = wT_load(q)
                for v in range(BL):
                    conv(q, v, wTq)
                    if q == 1 and v < 2:
                        w_prep(q + 1 + v)  # prep quarters 2,3 during q1 conv
                    if q == 3:
                        pack = tk.tile([42, T], F32, tag="pack")
                        stage2(v, pack)
                        topk(v, pack)

        for _rep in range(int(os.environ.get('BASS_NREP', '1'))):
            emit_once(_rep)

    nc.compile()
    return nc


_NC_CACHE = None


def _get_nc():
    global _NC_CACHE
    if _NC_CACHE is None:
        _NC_CACHE = build_nc()
    return _NC_CACHE


def make_in_maps(input_feature, conv_w, conv_b, att_w, att_b, cls_w, cls_b):
    input_feature = np.ascontiguousarray(input_feature, dtype=np.float32)
    conv_w = np.ascontiguousarray(conv_w, dtype=np.float32)
    conv_b = np.asarray(conv_b, dtype=np.float32)
    att_w = np.asarray(att_w, dtype=np.float32).reshape(2, D)
    att_b = np.asarray(att_b, dtype=np.float32)
    cls_w = np.asarray(cls_w, dtype=np.float32)
    cls_b = np.asarray(cls_b, dtype=np.float32)

    cmb = np.concatenate(
        [cls_w, att_w, (att_w[0] - att_w[1])[None, :]], axis=0)  # [24, D]
    cmbt = np.ascontiguousarray(cmb.T)  # [D, 24]
    zbias = np.concatenate(
        [cls_b, att_b, np.array([att_b[0] - att_b[1]], np.float32)]
    ).reshape(NCR, 1).astype(np.float32)
    cbias = np.ascontiguousarray(conv_b.reshape(NDT, 128).T)  # [128, 16]
    idm = np.eye(128, dtype=np.float32)
    kv = np.concatenate([np.full(NCLS, FGK), np.full(NCLS, BGK)]
                        ).reshape(42, 1).astype(np.float32)
    ki = np.concatenate([np.full(NCLS, 1.0 / FGK), np.full(NCLS, 1.0 / BGK)]
                        ).reshape(42, 1).astype(np.float32)

    in_maps = []
    for i in range(NCORES):
        in_maps.append({
            "x": np.ascontiguousarray(input_feature[i * BL:(i + 1) * BL]),
            "conv_w": conv_w,
            "cbias": cbias,
            "cmbt": cmbt,
            "zbias": zbias,
            "ident": idm,
            "kvec": kv,
            "kinv": ki,
        })
    return in_maps


def gather(rs):
    fg_cls = np.concatenate([r["fg_cls"] for r in rs], axis=0)
    bg_cls = np.concatenate([r["bg_cls"] for r in rs], axis=0)
    temp_att = np.concatenate([r["temp_att"] for r in rs], axis=0)
    cas_sm = np.concatenate([r["cas_sm"] for r in rs], axis=0)
    fg_sm = np.concatenate([r["fg_sm"] for r in rs], axis=0)
    bg_sm = np.concatenate([r["bg_sm"] for r in rs], axis=0)
    return (fg_cls, bg_cls, temp_att, cas_sm, fg_sm, bg_sm)


def kernel(input_feature, conv_w, conv_b, att_w, att_b, cls_w, cls_b):
    nc = _get_nc()
    in_maps = make_in_maps(input_feature, conv_w, conv_b, att_w, att_b,
                           cls_w, cls_b)
    res = run_bass_kernel_spmd(nc, in_maps, list(range(NCORES)))
    return gather(res.results)
